# revision 9
# baseline (speedup 1.0000x reference)
"""Trainium2 Bass kernel for nn_AdvancedQuantumFeatureMap.

Math (B=16384, Q=1024, F=2):
  amp  = L3(tanh(LN2(L2(gelu(LN1(L1(x)))))))       4096 -> 2048 -> 1024
  phase= tanh(P2(silu(LNp(P1(x)))))                2048 -> 1024
  qs   = (sin(f0*amp+p0) + cos(f1*phase+p1) + tanh(p2)) / 3
  out  = (qs @ Wv.T + bv) @ Wo.T + bo              (attention with seq_len 1)

Device strategy: pure data parallel over 8 cores (batch shard 2048/core),
transposed layout (features on partitions, batch on free dim), fp16 matmul
operands with fp32 PSUM accumulation.

Algebraic folds done on host (numpy):
 - LN1/LNp mean+var are closed-form in the 2-dim input -> folded into an
   augmented input  x_aug = [x0*inv, x1*inv, inv, -m*inv]  (K=4 matmul).
 - LN2 mean is linear in h1 -> one extra M=1 matmul row; centering via a
   K=1 matmul row; var = colsum(square(centered))/2048 via ones-matmul.
 - attention collapses to one matmul: Wc = attn_out_w @ Wv / 3, with rz
   (input-independent) and all biases folded into a bias row.
 - rotation scales f0/p0 folded into W3/b3; cos(z) = sin(z + pi/2).
"""

import numpy as np
from contextlib import ExitStack

import concourse.bass as bass
import concourse.tile as tile
from concourse import bacc, mybir
from concourse.bass_utils import run_bass_kernel_spmd

AF = mybir.ActivationFunctionType
OP = mybir.AluOpType
F16 = mybir.dt.float16
F32 = mybir.dt.float32
I32 = mybir.dt.int32
TWO_PI = 2.0 * float(np.pi)

B, Q, F = 16384, 1024, 2
NCORES = 8
BC = B // NCORES            # 2048 batch rows per core
NT = 512                    # batch-tile (matmul free dim)
NTILES = BC // NT           # 4
EPS = 1e-5

MC1A, MC1P = 32, 16         # L1 output chunks (4096, 2048)
MC2, KC2 = 16, 32           # L2: 2048 out, 4096 contraction
MC3, KC3 = 8, 16            # L3: 1024 out, 2048 contraction
MCP, KCP = 8, 16            # phase L2: 1024 out, 2048 contraction
MCF, KCF = 8, 8             # final: 1024 out, 1024 contraction

_BUILT = None


def _build(reps=1):
    nc = bacc.Bacc("TRN2", target_bir_lowering=False, debug=False,
                   num_devices=NCORES)

    def din(name, shape, dtype=F16):
        return nc.dram_tensor(name, list(shape), dtype,
                              kind="ExternalInput").ap()

    d_xaugA = din("xaugA", (4, BC))
    d_xaugP = din("xaugP", (4, BC))
    d_w1a = din("w1aT", (4, MC1A * 128))
    d_w1p = din("w1pT", (4, MC1P * 128))
    d_w2 = din("w2p", (128, MC2, KC2 * 128))
    d_cm2 = din("cm2T", (128, KC2))
    d_b2r = din("b2row", (1, MC2 * 128))
    d_w3 = din("w3p", (128, MC3, KC3 * 128))
    d_b3r = din("b3row", (1, MC3 * 128))
    d_wp = din("wpp", (128, MCP, KCP * 128))
    d_pbr = din("pbrow", (1, MCP * 128))
    d_wc = din("wcp", (128, MCF, KCF * 128))
    d_bcr = din("bcrow", (1, MCF * 128))
    d_g1 = din("g1T", (128, MC1A), F32)
    d_be1 = din("be1T", (128, MC1A), F32)
    d_gp = din("gpT", (128, MC1P), F32)
    d_bep = din("bepT", (128, MC1P), F32)
    d_g2 = din("g2T", (128, MC2), F32)
    d_be2 = din("be2T", (128, MC2), F32)
    d_f1 = din("f1T", (128, MCP), F32)
    d_p1c = din("p1cT", (128, MCP), F32)
    d_out = nc.dram_tensor("outT", [MCF * 128, BC], F32,
                           kind="ExternalOutput").ap()

    with tile.TileContext(nc) as tc, ExitStack() as ctx:
        def pool(name, bufs, space="SBUF"):
            return ctx.enter_context(
                tc.tile_pool(name=name, bufs=bufs, space=space))

        cst = pool("cst", 1)
        w2_p = pool("w2c", 2)
        w3_p = pool("w3c", 2)
        wp_p = pool("wpc", 2)
        wc_p = pool("wcc", 2)
        xa_p = pool("xap", 4)
        h1_p = pool("h1p", 1)
        a2_p = pool("a2p", 1)
        b16_p = pool("b16", 2)      # p1 / h2 share (16KB/p each)
        b8_p = pool("b8", 4)        # phase / ry / rx share (8KB/p each)
        sq_p = pool("sqp", 3)
        tn_p = pool("tnp", 3)
        rr_p = pool("rrp", 4)
        os_p = pool("osp", 3)
        st_p = pool("stp", 2)
        mm_ps = pool("mmps", 4, "PSUM")
        st_ps = pool("stps", 2, "PSUM")
        bc_ps = pool("bcps", 1, "PSUM")

        def ctile(dram, shape, dtype=F16, tg=None):
            t = cst.tile(shape, dtype, tag=tg, name=tg)
            nc.sync.dma_start(t[:], dram[:])
            return t

        w1a = ctile(d_w1a, (4, MC1A * 128), tg="w1a")
        w1p = ctile(d_w1p, (4, MC1P * 128), tg="w1p")
        cm2 = ctile(d_cm2, (128, KC2), tg="cm2")
        b2r = ctile(d_b2r, (1, MC2 * 128), tg="b2r")
        b3r = ctile(d_b3r, (1, MC3 * 128), tg="b3r")
        pbr = ctile(d_pbr, (1, MCP * 128), tg="pbr")
        bcr = ctile(d_bcr, (1, MCF * 128), tg="bcr")
        g1 = ctile(d_g1, (128, MC1A), F32, tg="g1")
        be1 = ctile(d_be1, (128, MC1A), F32, tg="be1")
        gp = ctile(d_gp, (128, MC1P), F32, tg="gp")
        bep = ctile(d_bep, (128, MC1P), F32, tg="bep")
        g2 = ctile(d_g2, (128, MC2), F32, tg="g2")
        be2 = ctile(d_be2, (128, MC2), F32, tg="be2")
        f1 = ctile(d_f1, (128, MCP), F32, tg="f1")
        p1c = ctile(d_p1c, (128, MCP), F32, tg="p1c")

        ones1 = cst.tile([1, NT], F16, tag="ones1", name="ones1")
        nc.vector.memset(ones1[:], 1.0)
        neg1 = cst.tile([1, 128], F16, tag="neg1", name="neg1")
        nc.vector.memset(neg1[:], -1.0)
        onesP = cst.tile([128, 1], F16, tag="onesP", name="onesP")
        nc.vector.memset(onesP[:], 1.0)
        onesK = cst.tile([1, 128], F16, tag="onesK", name="onesK")
        nc.vector.memset(onesK[:], 1.0)
        epsb = cst.tile([1, 1], F32, tag="epsb", name="epsb")
        nc.vector.memset(epsb[:], EPS)

        def stage_A(t):
            """Load inputs, L1 both encoders, phase L2, ry."""
            xaP = xa_p.tile([4, NT], F16, tag="xa", name="xaP")
            nc.sync.dma_start(xaP[:], d_xaugP[:, t * NT:(t + 1) * NT])
            xaA = xa_p.tile([4, NT], F16, tag="xa", name="xaA")
            nc.sync.dma_start(xaA[:], d_xaugA[:, t * NT:(t + 1) * NT])

            p1 = b16_p.tile([128, MC1P, NT], F16, tag="b16", name="p1")
            for m in range(MC1P):
                ps = mm_ps.tile([128, NT], F32, tag="mm", name="psL1p")
                nc.tensor.matmul(ps[:], w1p[:, m * 128:(m + 1) * 128],
                                 xaP[:], start=True, stop=True)
                nc.scalar.activation(p1[:, m, :], ps[:], AF.Silu,
                                     bias=bep[:, m:m + 1], scale=gp[:, m:m + 1])

            h1 = h1_p.tile([128, KC2, NT], F16, tag="h1", name="h1")
            for m in range(MC1A):
                ps = mm_ps.tile([128, NT], F32, tag="mm", name="psL1a")
                nc.tensor.matmul(ps[:], w1a[:, m * 128:(m + 1) * 128],
                                 xaA[:], start=True, stop=True)
                nc.scalar.activation(h1[:, m, :], ps[:], AF.Gelu,
                                     bias=be1[:, m:m + 1], scale=g1[:, m:m + 1])

            phase = b8_p.tile([128, MCP, NT], F16, tag="b8", name="phase")
            for m in range(MCP):
                wcol = wp_p.tile([128, KCP * 128], F16, tag="wp", name="wpcol")
                nc.sync.dma_start(wcol[:], d_wp[:, m, :])
                ps = mm_ps.tile([128, NT], F32, tag="mm", name="psP2")
                for kc in range(KCP):
                    nc.tensor.matmul(ps[:], wcol[:, kc * 128:(kc + 1) * 128],
                                     p1[:, kc, :], start=(kc == 0), stop=False)
                nc.tensor.matmul(ps[:], pbr[:, m * 128:(m + 1) * 128],
                                 ones1[:], start=False, stop=True)
                nc.scalar.activation(phase[:, m, :], ps[:], AF.Tanh)

            # ry = sin(f1*phase + p1c), range-reduced to [-pi, pi]
            ry = b8_p.tile([128, MCP, NT], F16, tag="b8", name="ry")
            for m in range(MCP):
                u = rr_p.tile([128, NT], F32, tag="rr", name="u")
                nc.vector.tensor_scalar(u[:], phase[:, m, :],
                                        f1[:, m:m + 1], p1c[:, m:m + 1],
                                        op0=OP.mult, op1=OP.add)
                ki = rr_p.tile([128, NT], I32, tag="rr", name="ki")
                nc.vector.tensor_scalar_mul(ki[:], u[:], 1.0 / TWO_PI)
                kf = rr_p.tile([128, NT], F32, tag="rr", name="kf")
                nc.vector.tensor_copy(kf[:], ki[:])
                zt = rr_p.tile([128, NT], F32, tag="rr", name="zt")
                nc.vector.scalar_tensor_tensor(zt[:], kf[:], -TWO_PI, u[:],
                                               op0=OP.mult, op1=OP.add)
                nc.scalar.activation(ry[:, m, :], zt[:], AF.Sin)
            return h1, ry

        def stage_B(t, h1):
            """LN2 mean row, main L2 matmul with centering, squares."""
            m2ps = st_ps.tile([1, NT], F32, tag="st", name="m2ps")
            for kc in range(KC2):
                nc.tensor.matmul(m2ps[:], cm2[:, kc:kc + 1], h1[:, kc, :],
                                 start=(kc == 0), stop=(kc == KC2 - 1))
            m2sb = st_p.tile([1, NT], F16, tag="m2", name="m2sb")
            nc.scalar.copy(m2sb[:], m2ps[:])

            a2c = a2_p.tile([128, MC2, NT], F16, tag="a2c", name="a2c")
            ssps = st_ps.tile([1, NT], F32, tag="st", name="ssps")
            sqs = []
            for m in range(MC2):
                wcol = w2_p.tile([128, KC2 * 128], F16, tag="w2", name="w2col")
                nc.sync.dma_start(wcol[:], d_w2[:, m, :])
                ps = mm_ps.tile([128, NT], F32, tag="mm", name="psL2")
                for kc in range(KC2):
                    nc.tensor.matmul(ps[:], wcol[:, kc * 128:(kc + 1) * 128],
                                     h1[:, kc, :], start=(kc == 0), stop=False)
                nc.tensor.matmul(ps[:], b2r[:, m * 128:(m + 1) * 128],
                                 ones1[:], start=False, stop=False)
                nc.tensor.matmul(ps[:], neg1[:], m2sb[:],
                                 start=False, stop=True)
                nc.vector.tensor_copy(a2c[:, m, :], ps[:])
                sq = sq_p.tile([128, NT], F16, tag="sq", name="sq")
                nc.scalar.activation(sq[:], ps[:], AF.Square)
                sqs.append(sq)
                if m >= 1:
                    nc.tensor.matmul(ssps[:], onesP[:], sqs[m - 1][:],
                                     start=(m == 1), stop=False)
            nc.tensor.matmul(ssps[:], onesP[:], sqs[MC2 - 1][:],
                             start=False, stop=True)
            return a2c, ssps

        def stage_C(t, a2c, ssps, ry):
            """Inverse-std, normalize+tanh, L3+sin, qs, final matmul, out."""
            s_sb = st_p.tile([1, NT], F32, tag="ssb", name="s_sb")
            nc.scalar.activation(s_sb[:], ssps[:], AF.Sqrt,
                                 bias=epsb[:], scale=1.0 / (MC2 * 128))
            inv_sb = st_p.tile([1, NT], F32, tag="ssb", name="inv_sb")
            nc.vector.reciprocal(inv_sb[:], s_sb[:])
            invhi = st_p.tile([1, NT], F16, tag="ivh", name="invhi")
            nc.vector.tensor_copy(invhi[:], inv_sb[:])
            invlo = st_p.tile([1, NT], F16, tag="ivh", name="invlo")
            nc.vector.scalar_tensor_tensor(invlo[:], inv_sb[:], 1.0,
                                           invhi[:], op0=OP.mult,
                                           op1=OP.subtract)
            invb = bc_ps.tile([128, NT], F32, tag="bc", name="invb")
            nc.tensor.matmul(invb[:], onesK[:], invhi[:],
                             start=True, stop=False)
            nc.tensor.matmul(invb[:], onesK[:], invlo[:],
                             start=False, stop=True)

            h2 = b16_p.tile([128, MC2, NT], F16, tag="b16", name="h2")
            for m in range(MC2):
                tn = tn_p.tile([128, NT], F32, tag="tn", name="tn")
                nc.vector.tensor_mul(tn[:], a2c[:, m, :], invb[:])
                nc.scalar.activation(h2[:, m, :], tn[:], AF.Tanh,
                                     bias=be2[:, m:m + 1], scale=g2[:, m:m + 1])

            rx = b8_p.tile([128, MC3, NT], F16, tag="b8", name="rx")
            for m in range(MC3):
                wcol = w3_p.tile([128, KC3 * 128], F16, tag="w3", name="w3col")
                nc.sync.dma_start(wcol[:], d_w3[:, m, :])
                ps = mm_ps.tile([128, NT], F32, tag="mm", name="psL3")
                for kc in range(KC3):
                    nc.tensor.matmul(ps[:], wcol[:, kc * 128:(kc + 1) * 128],
                                     h2[:, kc, :], start=(kc == 0), stop=False)
                nc.tensor.matmul(ps[:], b3r[:, m * 128:(m + 1) * 128],
                                 ones1[:], start=False, stop=True)
                # rx = sin(psum), range-reduced to [-pi, pi]
                ki = rr_p.tile([128, NT], I32, tag="rr", name="kix")
                nc.vector.tensor_scalar_mul(ki[:], ps[:], 1.0 / TWO_PI)
                kf = rr_p.tile([128, NT], F32, tag="rr", name="kfx")
                nc.vector.tensor_copy(kf[:], ki[:])
                zt = rr_p.tile([128, NT], F32, tag="rr", name="ztx")
                nc.vector.scalar_tensor_tensor(zt[:], kf[:], -TWO_PI, ps[:],
                                               op0=OP.mult, op1=OP.add)
                nc.scalar.activation(rx[:, m, :], zt[:], AF.Sin)

            for m in range(MCF):
                nc.vector.tensor_add(rx[:, m, :], rx[:, m, :], ry[:, m, :])

            for m in range(MCF):
                wcol = wc_p.tile([128, KCF * 128], F16, tag="wc", name="wccol")
                nc.sync.dma_start(wcol[:], d_wc[:, m, :])
                ps = mm_ps.tile([128, NT], F32, tag="mm", name="psF")
                for kc in range(KCF):
                    nc.tensor.matmul(ps[:], wcol[:, kc * 128:(kc + 1) * 128],
                                     rx[:, kc, :], start=(kc == 0), stop=False)
                nc.tensor.matmul(ps[:], bcr[:, m * 128:(m + 1) * 128],
                                 ones1[:], start=False, stop=True)
                osb = os_p.tile([128, NT], F32, tag="o", name="osb")
                nc.scalar.copy(osb[:], ps[:])
                nc.sync.dma_start(
                    d_out[m * 128:(m + 1) * 128, t * NT:(t + 1) * NT], osb[:])

        # software pipeline: A0 B0 A1 C0 B1 A2 C1 B2 A3 C2 B3 C3
        carry = {}
        NITER = NTILES * reps
        for it in range(NITER):
            t = it % NTILES
            h1, ry = stage_A(t)
            if it >= 1:
                stage_C((it - 1) % NTILES, *carry[it - 1])
                del carry[it - 1]
            a2c, ssps = stage_B(t, h1)
            carry[it] = (a2c, ssps, ry)
        stage_C((NITER - 1) % NTILES, *carry[NITER - 1])

    nc.compile()
    return nc


def _get_built():
    global _BUILT
    if _BUILT is None:
        _BUILT = _build()
    return _BUILT


def _prep_weight(W, MCn, KCn):
    # (MCn*128, KCn*128) -> (128, MCn, KCn*128) with [p, m, kc*128+mi] =
    # W[m*128+mi, kc*128+p]
    r = W.reshape(MCn, 128, KCn, 128).transpose(3, 0, 2, 1)
    return np.ascontiguousarray(r).reshape(128, MCn, KCn * 128)


def _colT(v, n):
    # (n*128,) -> (128, n) with [p, c] = v[c*128+p]
    return np.ascontiguousarray(v.reshape(n, 128).T)


def kernel(**inputs):
    nc = _get_built()
    f64 = np.float64
    g = lambda k: np.asarray(inputs[k], dtype=f64)

    x = g("x")
    W1, b1 = g("amp_W1"), g("amp_b1")
    g1, be1 = g("amp_g1"), g("amp_be1")
    W2, b2 = g("amp_W2"), g("amp_b2")
    g2, be2 = g("amp_g2"), g("amp_be2")
    W3, b3 = g("amp_W3"), g("amp_b3")
    pW1, pb1 = g("ph_W1"), g("ph_b1")
    pg1, pbe1 = g("ph_g1"), g("ph_be1")
    pW2, pb2 = g("ph_W2"), g("ph_b2")
    rf, rp = g("rot_freq"), g("rot_phase")
    aiw, aib = g("attn_in_w"), g("attn_in_b")
    aow, aob = g("attn_out_w"), g("attn_out_b")

    def ln1_aug(W, b):
        n = W.shape[0]
        m = x @ W.mean(0) + b.mean()
        s2 = ((x @ (W.T @ W / n)) * x).sum(1) + 2.0 * (x @ (W.T @ b / n)) \
            + (b * b).mean()
        inv = 1.0 / np.sqrt(np.maximum(s2 - m * m, 0.0) + EPS)
        return np.stack([x[:, 0] * inv, x[:, 1] * inv, inv, -m * inv], 0)

    xaugA = ln1_aug(W1, b1).astype(np.float16)
    xaugP = ln1_aug(pW1, pb1).astype(np.float16)

    w1aT = np.stack([W1[:, 0], W1[:, 1], b1, np.ones(4 * Q)], 0)
    w1pT = np.stack([pW1[:, 0], pW1[:, 1], pb1, np.ones(2 * Q)], 0)

    f0, p0 = rf[-1, :, 0], rp[-1, :, 0]
    f1v, p1cv = rf[-1, :, 1], rp[-1, :, 1] + np.pi / 2.0
    rz = np.tanh(rp[-1, :, 2])
    W3p = f0[:, None] * W3
    b3p = f0 * b3 + p0
    Wv, bv = aiw[2 * Q:], aib[2 * Q:]
    Wc = (aow @ Wv) / 3.0
    bc = Wc @ rz + aow @ bv + aob

    fp16 = np.float16
    in_common = {
        "w1aT": w1aT.astype(fp16), "w1pT": w1pT.astype(fp16),
        "w2p": _prep_weight(W2, MC2, KC2).astype(fp16),
        "cm2T": _colT(W2.mean(0), KC2).astype(fp16),
        "b2row": (b2 - b2.mean())[None, :].astype(fp16),
        "w3p": _prep_weight(W3p, MC3, KC3).astype(fp16),
        "b3row": b3p[None, :].astype(fp16),
        "wpp": _prep_weight(pW2, MCP, KCP).astype(fp16),
        "pbrow": pb2[None, :].astype(fp16),
        "wcp": _prep_weight(Wc, MCF, KCF).astype(fp16),
        "bcrow": bc[None, :].astype(fp16),
        "g1T": _colT(g1, MC1A).astype(np.float32),
        "be1T": _colT(be1, MC1A).astype(np.float32),
        "gpT": _colT(pg1, MC1P).astype(np.float32),
        "bepT": _colT(pbe1, MC1P).astype(np.float32),
        "g2T": _colT(g2, MC2).astype(np.float32),
        "be2T": _colT(be2, MC2).astype(np.float32),
        "f1T": _colT(f1v, MCP).astype(np.float32),
        "p1cT": _colT(p1cv, MCP).astype(np.float32),
    }
    in_maps = []
    for c in range(NCORES):
        m = dict(in_common)
        m["xaugA"] = np.ascontiguousarray(xaugA[:, c * BC:(c + 1) * BC])
        m["xaugP"] = np.ascontiguousarray(xaugP[:, c * BC:(c + 1) * BC])
        in_maps.append(m)

    res = run_bass_kernel_spmd(nc, in_maps, core_ids=list(range(NCORES)))
    out = np.empty((B, Q), np.float32)
    for c in range(NCORES):
        out[c * BC:(c + 1) * BC] = res.results[c]["outT"].T
    return out
